# revision 22
# baseline (speedup 1.0000x reference)
"""YOLO-style detector decode kernel for Trainium2, 8-core data-parallel.

kernel(**inputs) takes the full unsharded inputs (as produced by
setup_inputs) and returns (boxes [N,6] f32, valid [N] bool), matching the
reference. Batch is sharded 4-per-core across 8 NeuronCores; each core
decodes its 4 batches for all three scales with no cross-core
communication; host re-concatenates per-scale blocks.

Per-core pipeline (per batch x scale group):
  - DMA in two channel granules [128, HW] + [127, HW] (padded to K*128
    spatial cols)
  - PE transposes each 128-spatial chunk into PSUM [128, 255]
    (channels on the free axis)
  - class max m via DVE reduce_max on the transposed tiles (batched over
    4-chunk PSUM tiles)
  - DVE scalar_tensor_tensor per (chunk, anchor): (v >= m) * 2^-c,
    accum-summed -> x whose float32 exponent encodes the first argmax
    index exactly (ties sum into lower-order bits, leading exponent stays
    at the smallest class index)
  - cls = -((bitcast_i32(x) >> 23) - 127)
  - box fields: batched ACT sigmoid/exp + batched DVE ops build
    [conf, x1, y1, x2, y2, cls]; objectness mask compares raw logits
    against logit(thresh) so the bool mask is bit-exact vs the reference
"""

import math
import sys

import numpy as np

for _p in ("/opt/trn_rl_repo",):
    if _p not in sys.path:
        sys.path.insert(0, _p)

import bass_rust
import concourse.bass as bass
import concourse.mybir as mybir
from concourse.bass_utils import run_bass_kernel_spmd
from concourse.tile import TileContext

F32 = mybir.dt.float32
BF16 = mybir.dt.bfloat16
I32 = mybir.dt.int32
U8 = mybir.dt.uint8
BIGSCALE = float(2.0 ** 120)
# ACT-side extraction route: net-negative on the cost model (accumulate
# still runs on DVE); keep the pure-DVE path
ACT_ROUTE = False

B = 32
NCORES = 8
BL = B // NCORES  # 4 batches per core
CH = 255
NCLS = 80
# (side, stride, HW, K chunks of 128 spatial)
SCALES = [(13, 32.0, 169, 2), (26, 16.0, 676, 6), (52, 8.0, 2704, 22)]
ROWS_PER_CORE = BL * sum(hw for _, _, hw, _ in SCALES) * 3  # 42588
# DRAM outputs are padded to whole 128-spatial chunks per (batch, scale)
ROWS_PAD = BL * sum(k * 384 for _, _, _, k in SCALES)  # 46080


def _build_consts(anchors_by_scale):
    """Host-side constant tensors shared by all cores."""
    consts = {}
    consts["ident"] = np.eye(128, dtype=np.float32)
    # 2^-c weights, replicated on all 128 partitions
    w = (2.0 ** -np.arange(NCLS, dtype=np.float64)).astype(np.float32)
    consts["wexact"] = np.broadcast_to(w, (128, NCLS)).copy()
    import ml_dtypes
    consts["wexb"] = consts["wexact"].astype(ml_dtypes.bfloat16)
    for si, (side, t, hw, k) in enumerate(SCALES):
        g = np.zeros((128, k * 6), dtype=np.float32)
        awh = np.zeros((128, k * 6), dtype=np.float32)
        anc = anchors_by_scale[si]  # [3,2]
        for kk in range(k):
            for p in range(128):
                s = kk * 128 + p
                if s >= hw:
                    continue
                gx = float(s % side) * t
                gy = float(s // side) * t
                for a in range(3):
                    g[p, kk * 6 + a * 2 + 0] = gx
                    g[p, kk * 6 + a * 2 + 1] = gy
        for a in range(3):
            awh[:, a * 2 + 0 :: 6] = float(anc[a, 0]) * 0.5
            awh[:, a * 2 + 1 :: 6] = float(anc[a, 1]) * 0.5
        consts[f"grid{side}"] = g
        consts[f"awh{side}"] = awh
    return consts


def _legalize_waits(nc, maxw=1):
    """HW ISA instructions have a small number of sync-wait slots; Tile can
    emit more. Hoist excess waits onto same-engine NoOps placed immediately
    before the instruction (engine program order preserves semantics)."""
    n = 0
    for fn in nc.m.functions:
        for blk in fn.blocks:
            li = blk.instructions
            out = []
            for inst in li:
                maxw_i = maxw
                si = inst.sync_info
                waits = list(si.on_wait) if si is not None and si.on_wait else []
                if len(waits) > maxw_i:
                    hoisted, keep = waits[: len(waits) - maxw_i], waits[-maxw_i:]
                    for j in range(0, len(hoisted), maxw):
                        nop = bass_rust.InstNoOp(
                            name=f"waitnop-{n}", ins=[], outs=[]
                        )
                        n += 1
                        nop.engine = inst.engine
                        nop.sync_info = mybir.SyncInfo(
                            on_wait=hoisted[j : j + maxw], on_update=[]
                        )
                        out.append(nop)
                    inst.sync_info = mybir.SyncInfo(
                        on_wait=keep,
                        on_update=list(si.on_update) if si.on_update else [],
                    )
                out.append(inst)
            li[:] = out


def _trace_kernel(nc, logit_thresh):
    """Emit the per-core BIR program. All cores run the same program."""
    xs = {}
    for side, _, hw, k in SCALES:
        # inputs are host-padded to whole 128-spatial chunks
        xs[side] = nc.declare_dram_parameter(f"x{side}", [BL, CH, k * 128], F32, isOutput=False)
    ident_d = nc.declare_dram_parameter("ident", [128, 128], F32, isOutput=False)
    wexact_d = nc.declare_dram_parameter("wexact", [128, NCLS], F32, isOutput=False)
    wexb_d = nc.declare_dram_parameter("wexb", [128, NCLS], BF16, isOutput=False)
    grid_d, awh_d = {}, {}
    for side, _, hw, k in SCALES:
        grid_d[side] = nc.declare_dram_parameter(f"grid{side}", [128, k * 6], F32, isOutput=False)
        awh_d[side] = nc.declare_dram_parameter(f"awh{side}", [128, k * 6], F32, isOutput=False)
    boxes_d = nc.declare_dram_parameter("boxes", [ROWS_PAD, 6], F32, isOutput=True)
    valid_d = nc.declare_dram_parameter("valid", [ROWS_PAD], U8, isOutput=True)

    AX = mybir.AxisListType.X
    OP = mybir.AluOpType
    ACTF = mybir.ActivationFunctionType

    with TileContext(nc) as tc:
        with (
            tc.tile_pool(name="consts", bufs=1) as cpool,
            tc.tile_pool(name="gran", bufs=3) as gpool,
            tc.tile_pool(name="work", bufs=3) as wpool,
            tc.tile_pool(name="scr", bufs=4) as spool,
            tc.tile_pool(name="psum", bufs=4, space="PSUM") as ppool,
        ):
            ident = cpool.tile([128, 128], F32, tag="ident")
            nc.sync.dma_start(out=ident[:, :], in_=ident_d[:, :])
            wexact = cpool.tile([128, NCLS], F32, tag="wex")
            nc.sync.dma_start(out=wexact[:, :], in_=wexact_d[:, :])
            wexb = cpool.tile([128, NCLS], BF16, tag="wexb")
            nc.sync.dma_start(out=wexb[:, :], in_=wexb_d[:, :])
            grids, awhs = {}, {}
            for side, _, hw, k in SCALES:
                grids[side] = cpool.tile(
                    [128, k * 6], F32, tag=f"grid{side}", name=f"grid{side}"
                )
                nc.sync.dma_start(out=grids[side][:, :], in_=grid_d[side][:, :])
                awhs[side] = cpool.tile(
                    [128, k * 6], F32, tag=f"awh{side}", name=f"awh{side}"
                )
                nc.sync.dma_start(out=awhs[side][:, :], in_=awh_d[side][:, :])

            row_base = 0  # running row offset into the per-core output
            for side, t, hw, K in SCALES:
                KP = K * 128
                for b in range(BL):
                    # --- channel-major granules (input pre-padded on host) ---
                    g0 = gpool.tile([128, KP], F32, tag=f"g0_{side}")
                    g1 = gpool.tile([127, KP], F32, tag=f"g1_{side}")
                    nc.sync.dma_start(out=g0[:, :], in_=xs[side][b, 0:128, :])
                    nc.sync.dma_start(out=g1[:, :], in_=xs[side][b, 128:255, :])

                    sxyc = wpool.tile([128, K * 9], F32, tag=f"sxyc{side}")
                    ewh = wpool.tile([128, K * 6], F32, tag=f"ewh{side}")
                    cxy = wpool.tile([128, K * 6], F32, tag=f"cxy{side}")
                    mmax = wpool.tile([128, K * 3], F32, tag=f"m{side}")
                    negm2 = wpool.tile([128, K * 3], F32, tag=f"nm{side}")
                    xacc = wpool.tile([128, K * 3], F32, tag=f"x{side}")
                    clsi = wpool.tile([128, K * 3], I32, tag=f"ci{side}")
                    otile = wpool.tile([128, K * 18], F32, tag=f"o{side}")
                    vtile = wpool.tile([128, K * 3], U8, tag=f"v{side}")

                    nblk = (K + 3) // 4
                    for bk in range(nblk):
                        k0 = bk * 4
                        nb = min(4, K - k0)
                        tp = ppool.tile([128, 1024], F32, tag="tp")
                        for kk in range(k0, k0 + nb):
                            cb = (kk - k0) * 256
                            s0 = kk * 128
                            sl = slice(s0, s0 + 128)
                            nc.tensor.transpose(
                                tp[:, cb : cb + 128], g0[:, sl], ident[0:128, 0:128]
                            )
                            nc.tensor.transpose(
                                tp[:, cb + 128 : cb + 255], g1[:, sl], ident[0:127, 0:127]
                            )
                        tca = tp[:, :].rearrange("p (c x) -> p c x", c=4)
                        t4 = tca[:, 0:nb, 0:255].rearrange(
                            "p c (a f) -> p c a f", a=3
                        )  # [128,nb,3,85]
                        tbox = t4[:, :, :, 0:5]
                        tcls = t4[:, :, :, 5:85]
                        m3 = mmax[:, :].rearrange("p (k a) -> p k a", a=3)
                        nc.vector.reduce_max(
                            out=m3[:, k0 : k0 + nb, :], in_=tcls, axis=AX
                        )
                        if ACT_ROUTE and side == 52:
                            # bias for the ACT-side extraction: -m * 2^120
                            nc.vector.tensor_scalar(
                                out=negm2[:, k0 * 3 : (k0 + nb) * 3],
                                in0=mmax[:, k0 * 3 : (k0 + nb) * 3],
                                scalar1=-BIGSCALE,
                                scalar2=None,
                                op0=OP.mult,
                            )
                        # objectness from raw logits, batched over the block
                        nc.vector.tensor_scalar(
                            out=vtile[:, :].rearrange("p (k a) -> p k a", a=3)[
                                :, k0 : k0 + nb, :
                            ],
                            in0=tbox[:, :, :, 0:1],
                            scalar1=logit_thresh,
                            scalar2=None,
                            op0=OP.is_gt,
                        )
                        # sigmoid of (conf, tx, ty); exp of (tw, th)
                        nc.scalar.activation(
                            out=sxyc[:, :].rearrange("p (k a d) -> p k a d", a=3, d=3)[
                                :, k0 : k0 + nb, :, :
                            ],
                            in_=tbox[:, :, :, 0:3],
                            func=ACTF.Sigmoid,
                        )
                        nc.scalar.activation(
                            out=ewh[:, :].rearrange("p (k a d) -> p k a d", a=3, d=2)[
                                :, k0 : k0 + nb, :, :
                            ],
                            in_=tbox[:, :, :, 3:5],
                            func=ACTF.Exp,
                        )
                        # (v >= m) * 2^-c, accum -> exponent encodes argmax
                        for kk in range(k0, k0 + nb):
                            cb = (kk - k0) * 256
                            for a in range(3):
                                col = kk * 3 + a
                                if ACT_ROUTE and side == 52 and a >= 1:
                                    # exact ACT route: q = sigmoid((v-m)*2^120)
                                    # is exactly 0.5 at the max, 0 elsewhere;
                                    # 2q * 2^-c accumulated on GPSIMD
                                    q = spool.tile(
                                        [128, NCLS], BF16, tag="q", name="q"
                                    )
                                    nc.scalar.activation(
                                        out=q[:, :],
                                        in_=tp[:, cb + 85 * a + 5 : cb + 85 * a + 85],
                                        func=ACTF.Sigmoid,
                                        bias=negm2[:, col : col + 1],
                                        scale=BIGSCALE,
                                    )
                                    scr2 = spool.tile(
                                        [128, NCLS], BF16, tag="scr2", name="scr2"
                                    )
                                    nc.vector.scalar_tensor_tensor(
                                        out=scr2[:, :],
                                        in0=q[:, :],
                                        scalar=2.0,
                                        in1=wexb[:, :],
                                        op0=OP.mult,
                                        op1=OP.mult,
                                        accum_out=xacc[:, col : col + 1],
                                    )
                                else:
                                    scr = spool.tile([128, NCLS], F32, tag="scr")
                                    nc.vector.scalar_tensor_tensor(
                                        out=scr[:, :],
                                        in0=tp[:, cb + 85 * a + 5 : cb + 85 * a + 85],
                                        scalar=mmax[:, col : col + 1],
                                        in1=wexact[:, :],
                                        op0=OP.is_ge,
                                        op1=OP.mult,
                                        accum_out=xacc[:, col : col + 1],
                                    )

                    # ---- batched per-group ops ----
                    s4 = sxyc[:, :].rearrange("p (k a d) -> p k a d", a=3, d=3)
                    o4 = otile[:, :].rearrange("p (k a f) -> p k a f", a=3, f=6)
                    c4 = cxy[:, :].rearrange("p (k a d) -> p k a d", a=3, d=2)
                    e4 = ewh[:, :].rearrange("p (k a d) -> p k a d", a=3, d=2)
                    # conf -> output
                    nc.gpsimd.tensor_copy(o4[:, :, :, 0:1], s4[:, :, :, 0:1])
                    # half extents: exp * (anchor/2)
                    nc.vector.tensor_tensor(
                        out=ewh[:, :], in0=ewh[:, :], in1=awhs[side][:, :], op=OP.mult
                    )
                    # centers: sig*t + grid*t
                    nc.vector.scalar_tensor_tensor(
                        out=cxy[:, :],
                        in0=s4[:, :, :, 1:3],
                        scalar=float(t),
                        in1=grids[side][:, :].rearrange(
                            "p (k a d) -> p k a d", a=3, d=2
                        ),
                        op0=OP.mult,
                        op1=OP.add,
                    )
                    nc.vector.tensor_tensor(
                        out=o4[:, :, :, 1:3], in0=c4, in1=e4, op=OP.subtract
                    )
                    nc.vector.tensor_tensor(
                        out=o4[:, :, :, 3:5], in0=c4, in1=e4, op=OP.add
                    )
                    # cls decode: exponent bits of xacc
                    nc.vector.tensor_scalar(
                        out=clsi[:, :],
                        in0=xacc[:, :].bitcast(I32),
                        scalar1=23,
                        scalar2=None,
                        op0=OP.logical_shift_right,
                    )
                    # cls = 127 - biased_exponent
                    nc.vector.tensor_scalar(
                        out=o4[:, :, :, 5:6],
                        in0=clsi[:, :].rearrange("p (k a) -> p k a", a=3).unsqueeze(3),
                        scalar1=-1,
                        scalar2=127,
                        op0=OP.mult,
                        op1=OP.add,
                    )

                    # ---- batched stores (chunk-padded DRAM layout) ----
                    rp = row_base + b * K * 384
                    nc.scalar.dma_start(
                        out=boxes_d[rp : rp + K * 384, :].rearrange(
                            "(k p a) f -> p k a f", p=128, a=3
                        ),
                        in_=otile[:, :].rearrange("p (k a f) -> p k a f", a=3, f=6),
                    )
                    nc.scalar.dma_start(
                        out=valid_d[rp : rp + K * 384].rearrange(
                            "(k p a) -> p k a", p=128, a=3
                        ),
                        in_=vtile[:, :].rearrange("p (k a) -> p k a", a=3),
                    )
                row_base += BL * K * 384
    return nc


def run(output_13, output_26, output_52, anchors_13, anchors_26, anchors_52, thresh,
        **spmd_kwargs):
    """Builds + runs the 8-core kernel; returns ((boxes, valid), BassKernelResults)."""
    anchors = [np.asarray(anchors_13), np.asarray(anchors_26), np.asarray(anchors_52)]
    th = float(np.asarray(thresh))
    # sigmoid(v) > th  <=>  v > logit(th)
    logit_thresh = float(np.float32(math.log(th / (1.0 - th))))

    nc = bass.Bass()
    _trace_kernel(nc, logit_thresh)
    _legalize_waits(nc)

    consts = _build_consts(anchors)
    raw = {13: output_13, 26: output_26, 52: output_52}
    feats = {}
    for side, _, hw, k in SCALES:
        f = np.asarray(raw[side]).reshape(B, CH, hw)
        fp = np.zeros((B, CH, k * 128), dtype=np.float32)
        fp[:, :, :hw] = f
        feats[side] = fp
    in_maps = []
    for g in range(NCORES):
        m = {f"x{side}": np.ascontiguousarray(feats[side][g * BL : (g + 1) * BL])
             for side, _, _, _ in SCALES}
        m.update(consts)
        in_maps.append(m)

    br = run_bass_kernel_spmd(nc, in_maps, list(range(NCORES)), **spmd_kwargs)
    res = br.results

    # host gather: per-scale blocks, cores (= batch shards) in order,
    # stripping the per-(batch,scale) chunk padding
    boxes_parts, valid_parts = [], []
    off = 0
    for side, _, hw, k in SCALES:
        nv = hw * 3
        for g in range(NCORES):
            for b in range(BL):
                r0 = off + b * k * 384
                boxes_parts.append(res[g]["boxes"][r0 : r0 + nv])
                valid_parts.append(res[g]["valid"][r0 : r0 + nv])
        off += BL * k * 384
    boxes = np.concatenate(boxes_parts, axis=0)
    valid = np.concatenate(valid_parts, axis=0) != 0
    return (boxes, valid), br


def kernel(output_13, output_26, output_52, anchors_13, anchors_26, anchors_52, thresh):
    (boxes, valid), _ = run(
        output_13, output_26, output_52, anchors_13, anchors_26, anchors_52, thresh
    )
    return boxes, valid


if __name__ == "__main__":
    rng = np.random.default_rng(0)
    ins = {
        "output_13": rng.standard_normal((B, CH, 13, 13), dtype=np.float32),
        "output_26": rng.standard_normal((B, CH, 26, 26), dtype=np.float32),
        "output_52": rng.standard_normal((B, CH, 52, 52), dtype=np.float32),
        "anchors_13": np.array([[116.0, 90.0], [156.0, 198.0], [373.0, 326.0]], np.float32),
        "anchors_26": np.array([[30.0, 61.0], [62.0, 45.0], [59.0, 119.0]], np.float32),
        "anchors_52": np.array([[10.0, 13.0], [16.0, 30.0], [33.0, 23.0]], np.float32),
        "thresh": np.float32(0.6),
    }
    b, v = kernel(**ins)
    print(b.shape, v.shape, v.sum())


# revision 23
# speedup vs baseline: 1.0248x; 1.0248x over previous
"""YOLO-style detector decode kernel for Trainium2, 8-core data-parallel.

kernel(**inputs) takes the full unsharded inputs (as produced by
setup_inputs) and returns (boxes [N,6] f32, valid [N] bool), matching the
reference. Batch is sharded 4-per-core across 8 NeuronCores; each core
decodes its 4 batches for all three scales with no cross-core
communication; host re-concatenates per-scale blocks.

Per-core pipeline (per batch x scale group):
  - DMA in two channel granules [128, HW] + [127, HW] (padded to K*128
    spatial cols)
  - PE transposes each 128-spatial chunk into PSUM [128, 255]
    (channels on the free axis)
  - class max m via DVE reduce_max on the transposed tiles (batched over
    4-chunk PSUM tiles)
  - DVE scalar_tensor_tensor per (chunk, anchor): (v >= m) * 2^-c,
    accum-summed -> x whose float32 exponent encodes the first argmax
    index exactly (ties sum into lower-order bits, leading exponent stays
    at the smallest class index)
  - cls = -((bitcast_i32(x) >> 23) - 127)
  - box fields: batched ACT sigmoid/exp + batched DVE ops build
    [conf, x1, y1, x2, y2, cls]; objectness mask compares raw logits
    against logit(thresh) so the bool mask is bit-exact vs the reference
"""

import math
import sys

import numpy as np

for _p in ("/opt/trn_rl_repo",):
    if _p not in sys.path:
        sys.path.insert(0, _p)

import bass_rust
import concourse.bass as bass
import concourse.mybir as mybir
from concourse.bass_utils import run_bass_kernel_spmd
from concourse.tile import TileContext

F32 = mybir.dt.float32
BF16 = mybir.dt.bfloat16
I32 = mybir.dt.int32
U8 = mybir.dt.uint8
BIGSCALE = float(2.0 ** 120)
# ACT-side extraction route: net-negative on the cost model (accumulate
# still runs on DVE); keep the pure-DVE path
ACT_ROUTE = False
# chunks per PSUM tile (tile free size = CPT*256 f32); PSUM bufs chosen so
# CPT * bufs * 256 * 4B <= 16KB/partition
CPT = 2
PSUM_BUFS = 8

B = 32
NCORES = 8
BL = B // NCORES  # 4 batches per core
CH = 255
NCLS = 80
# (side, stride, HW, K chunks of 128 spatial)
SCALES = [(13, 32.0, 169, 2), (26, 16.0, 676, 6), (52, 8.0, 2704, 22)]
ROWS_PER_CORE = BL * sum(hw for _, _, hw, _ in SCALES) * 3  # 42588
# DRAM outputs are padded to whole 128-spatial chunks per (batch, scale)
ROWS_PAD = BL * sum(k * 384 for _, _, _, k in SCALES)  # 46080


def _build_consts(anchors_by_scale):
    """Host-side constant tensors shared by all cores."""
    consts = {}
    consts["ident"] = np.eye(128, dtype=np.float32)
    # 2^-c weights, replicated on all 128 partitions
    w = (2.0 ** -np.arange(NCLS, dtype=np.float64)).astype(np.float32)
    consts["wexact"] = np.broadcast_to(w, (128, NCLS)).copy()
    import ml_dtypes
    consts["wexb"] = consts["wexact"].astype(ml_dtypes.bfloat16)
    for si, (side, t, hw, k) in enumerate(SCALES):
        g = np.zeros((128, k * 6), dtype=np.float32)
        awh = np.zeros((128, k * 6), dtype=np.float32)
        anc = anchors_by_scale[si]  # [3,2]
        for kk in range(k):
            for p in range(128):
                s = kk * 128 + p
                if s >= hw:
                    continue
                gx = float(s % side) * t
                gy = float(s // side) * t
                for a in range(3):
                    g[p, kk * 6 + a * 2 + 0] = gx
                    g[p, kk * 6 + a * 2 + 1] = gy
        for a in range(3):
            awh[:, a * 2 + 0 :: 6] = float(anc[a, 0]) * 0.5
            awh[:, a * 2 + 1 :: 6] = float(anc[a, 1]) * 0.5
        consts[f"grid{side}"] = g
        consts[f"awh{side}"] = awh
    return consts


def _legalize_waits(nc, maxw=1):
    """HW ISA instructions have a small number of sync-wait slots; Tile can
    emit more. Hoist excess waits onto same-engine NoOps placed immediately
    before the instruction (engine program order preserves semantics)."""
    n = 0
    for fn in nc.m.functions:
        for blk in fn.blocks:
            li = blk.instructions
            out = []
            for inst in li:
                maxw_i = maxw
                si = inst.sync_info
                waits = list(si.on_wait) if si is not None and si.on_wait else []
                if len(waits) > maxw_i:
                    hoisted, keep = waits[: len(waits) - maxw_i], waits[-maxw_i:]
                    for j in range(0, len(hoisted), maxw):
                        nop = bass_rust.InstNoOp(
                            name=f"waitnop-{n}", ins=[], outs=[]
                        )
                        n += 1
                        nop.engine = inst.engine
                        nop.sync_info = mybir.SyncInfo(
                            on_wait=hoisted[j : j + maxw], on_update=[]
                        )
                        out.append(nop)
                    inst.sync_info = mybir.SyncInfo(
                        on_wait=keep,
                        on_update=list(si.on_update) if si.on_update else [],
                    )
                out.append(inst)
            li[:] = out


def _trace_kernel(nc, logit_thresh):
    """Emit the per-core BIR program. All cores run the same program."""
    xs = {}
    for side, _, hw, k in SCALES:
        # inputs are host-padded to whole 128-spatial chunks
        xs[side] = nc.declare_dram_parameter(f"x{side}", [BL, CH, k * 128], F32, isOutput=False)
    ident_d = nc.declare_dram_parameter("ident", [128, 128], F32, isOutput=False)
    wexact_d = nc.declare_dram_parameter("wexact", [128, NCLS], F32, isOutput=False)
    wexb_d = nc.declare_dram_parameter("wexb", [128, NCLS], BF16, isOutput=False)
    grid_d, awh_d = {}, {}
    for side, _, hw, k in SCALES:
        grid_d[side] = nc.declare_dram_parameter(f"grid{side}", [128, k * 6], F32, isOutput=False)
        awh_d[side] = nc.declare_dram_parameter(f"awh{side}", [128, k * 6], F32, isOutput=False)
    boxes_d = nc.declare_dram_parameter("boxes", [ROWS_PAD, 6], F32, isOutput=True)
    valid_d = nc.declare_dram_parameter("valid", [ROWS_PAD], U8, isOutput=True)

    AX = mybir.AxisListType.X
    OP = mybir.AluOpType
    ACTF = mybir.ActivationFunctionType

    with TileContext(nc) as tc:
        with (
            tc.tile_pool(name="consts", bufs=1) as cpool,
            tc.tile_pool(name="gran", bufs=3) as gpool,
            tc.tile_pool(name="work", bufs=3) as wpool,
            tc.tile_pool(name="scr", bufs=4) as spool,
            tc.tile_pool(name="psum", bufs=PSUM_BUFS, space="PSUM") as ppool,
        ):
            ident = cpool.tile([128, 128], F32, tag="ident")
            nc.sync.dma_start(out=ident[:, :], in_=ident_d[:, :])
            wexact = cpool.tile([128, NCLS], F32, tag="wex")
            nc.sync.dma_start(out=wexact[:, :], in_=wexact_d[:, :])
            wexb = cpool.tile([128, NCLS], BF16, tag="wexb")
            nc.sync.dma_start(out=wexb[:, :], in_=wexb_d[:, :])
            grids, awhs = {}, {}
            for side, _, hw, k in SCALES:
                grids[side] = cpool.tile(
                    [128, k * 6], F32, tag=f"grid{side}", name=f"grid{side}"
                )
                nc.sync.dma_start(out=grids[side][:, :], in_=grid_d[side][:, :])
                awhs[side] = cpool.tile(
                    [128, k * 6], F32, tag=f"awh{side}", name=f"awh{side}"
                )
                nc.sync.dma_start(out=awhs[side][:, :], in_=awh_d[side][:, :])

            row_base = 0  # running row offset into the per-core output
            for side, t, hw, K in SCALES:
                KP = K * 128
                for b in range(BL):
                    # --- channel-major granules (input pre-padded on host) ---
                    g0 = gpool.tile([128, KP], F32, tag=f"g0_{side}")
                    g1 = gpool.tile([127, KP], F32, tag=f"g1_{side}")
                    nc.sync.dma_start(out=g0[:, :], in_=xs[side][b, 0:128, :])
                    nc.sync.dma_start(out=g1[:, :], in_=xs[side][b, 128:255, :])

                    sxyc = wpool.tile([128, K * 9], F32, tag=f"sxyc{side}")
                    ewh = wpool.tile([128, K * 6], F32, tag=f"ewh{side}")
                    cxy = wpool.tile([128, K * 6], F32, tag=f"cxy{side}")
                    mmax = wpool.tile([128, K * 3], F32, tag=f"m{side}")
                    negm2 = wpool.tile([128, K * 3], F32, tag=f"nm{side}")
                    xacc = wpool.tile([128, K * 3], F32, tag=f"x{side}")
                    clsi = wpool.tile([128, K * 3], I32, tag=f"ci{side}")
                    otile = wpool.tile([128, K * 18], F32, tag=f"o{side}")
                    vtile = wpool.tile([128, K * 3], U8, tag=f"v{side}")

                    nblk = (K + CPT - 1) // CPT
                    for bk in range(nblk):
                        k0 = bk * CPT
                        nb = min(CPT, K - k0)
                        tp = ppool.tile([128, CPT * 256], F32, tag="tp")
                        for kk in range(k0, k0 + nb):
                            cb = (kk - k0) * 256
                            s0 = kk * 128
                            sl = slice(s0, s0 + 128)
                            nc.tensor.transpose(
                                tp[:, cb : cb + 128], g0[:, sl], ident[0:128, 0:128]
                            )
                            nc.tensor.transpose(
                                tp[:, cb + 128 : cb + 255], g1[:, sl], ident[0:127, 0:127]
                            )
                        tca = tp[:, :].rearrange("p (c x) -> p c x", c=CPT)
                        t4 = tca[:, 0:nb, 0:255].rearrange(
                            "p c (a f) -> p c a f", a=3
                        )  # [128,nb,3,85]
                        tbox = t4[:, :, :, 0:5]
                        tcls = t4[:, :, :, 5:85]
                        m3 = mmax[:, :].rearrange("p (k a) -> p k a", a=3)
                        nc.vector.reduce_max(
                            out=m3[:, k0 : k0 + nb, :], in_=tcls, axis=AX
                        )
                        if ACT_ROUTE and side == 52:
                            # bias for the ACT-side extraction: -m * 2^120
                            nc.vector.tensor_scalar(
                                out=negm2[:, k0 * 3 : (k0 + nb) * 3],
                                in0=mmax[:, k0 * 3 : (k0 + nb) * 3],
                                scalar1=-BIGSCALE,
                                scalar2=None,
                                op0=OP.mult,
                            )
                        # objectness from raw logits, batched over the block
                        nc.vector.tensor_scalar(
                            out=vtile[:, :].rearrange("p (k a) -> p k a", a=3)[
                                :, k0 : k0 + nb, :
                            ],
                            in0=tbox[:, :, :, 0:1],
                            scalar1=logit_thresh,
                            scalar2=None,
                            op0=OP.is_gt,
                        )
                        # sigmoid of (conf, tx, ty); exp of (tw, th)
                        nc.scalar.activation(
                            out=sxyc[:, :].rearrange("p (k a d) -> p k a d", a=3, d=3)[
                                :, k0 : k0 + nb, :, :
                            ],
                            in_=tbox[:, :, :, 0:3],
                            func=ACTF.Sigmoid,
                        )
                        nc.scalar.activation(
                            out=ewh[:, :].rearrange("p (k a d) -> p k a d", a=3, d=2)[
                                :, k0 : k0 + nb, :, :
                            ],
                            in_=tbox[:, :, :, 3:5],
                            func=ACTF.Exp,
                        )
                        # (v >= m) * 2^-c, accum -> exponent encodes argmax
                        for kk in range(k0, k0 + nb):
                            cb = (kk - k0) * 256
                            for a in range(3):
                                col = kk * 3 + a
                                if ACT_ROUTE and side == 52 and a >= 1:
                                    # exact ACT route: q = sigmoid((v-m)*2^120)
                                    # is exactly 0.5 at the max, 0 elsewhere;
                                    # 2q * 2^-c accumulated on GPSIMD
                                    q = spool.tile(
                                        [128, NCLS], BF16, tag="q", name="q"
                                    )
                                    nc.scalar.activation(
                                        out=q[:, :],
                                        in_=tp[:, cb + 85 * a + 5 : cb + 85 * a + 85],
                                        func=ACTF.Sigmoid,
                                        bias=negm2[:, col : col + 1],
                                        scale=BIGSCALE,
                                    )
                                    scr2 = spool.tile(
                                        [128, NCLS], BF16, tag="scr2", name="scr2"
                                    )
                                    nc.vector.scalar_tensor_tensor(
                                        out=scr2[:, :],
                                        in0=q[:, :],
                                        scalar=2.0,
                                        in1=wexb[:, :],
                                        op0=OP.mult,
                                        op1=OP.mult,
                                        accum_out=xacc[:, col : col + 1],
                                    )
                                else:
                                    scr = spool.tile([128, NCLS], F32, tag="scr")
                                    nc.vector.scalar_tensor_tensor(
                                        out=scr[:, :],
                                        in0=tp[:, cb + 85 * a + 5 : cb + 85 * a + 85],
                                        scalar=mmax[:, col : col + 1],
                                        in1=wexact[:, :],
                                        op0=OP.is_ge,
                                        op1=OP.mult,
                                        accum_out=xacc[:, col : col + 1],
                                    )

                    # ---- batched per-group ops ----
                    s4 = sxyc[:, :].rearrange("p (k a d) -> p k a d", a=3, d=3)
                    o4 = otile[:, :].rearrange("p (k a f) -> p k a f", a=3, f=6)
                    c4 = cxy[:, :].rearrange("p (k a d) -> p k a d", a=3, d=2)
                    e4 = ewh[:, :].rearrange("p (k a d) -> p k a d", a=3, d=2)
                    # conf -> output
                    nc.gpsimd.tensor_copy(o4[:, :, :, 0:1], s4[:, :, :, 0:1])
                    # half extents: exp * (anchor/2)
                    nc.vector.tensor_tensor(
                        out=ewh[:, :], in0=ewh[:, :], in1=awhs[side][:, :], op=OP.mult
                    )
                    # centers: sig*t + grid*t
                    nc.vector.scalar_tensor_tensor(
                        out=cxy[:, :],
                        in0=s4[:, :, :, 1:3],
                        scalar=float(t),
                        in1=grids[side][:, :].rearrange(
                            "p (k a d) -> p k a d", a=3, d=2
                        ),
                        op0=OP.mult,
                        op1=OP.add,
                    )
                    nc.vector.tensor_tensor(
                        out=o4[:, :, :, 1:3], in0=c4, in1=e4, op=OP.subtract
                    )
                    nc.vector.tensor_tensor(
                        out=o4[:, :, :, 3:5], in0=c4, in1=e4, op=OP.add
                    )
                    # cls decode: exponent bits of xacc
                    nc.vector.tensor_scalar(
                        out=clsi[:, :],
                        in0=xacc[:, :].bitcast(I32),
                        scalar1=23,
                        scalar2=None,
                        op0=OP.logical_shift_right,
                    )
                    # cls = 127 - biased_exponent
                    nc.vector.tensor_scalar(
                        out=o4[:, :, :, 5:6],
                        in0=clsi[:, :].rearrange("p (k a) -> p k a", a=3).unsqueeze(3),
                        scalar1=-1,
                        scalar2=127,
                        op0=OP.mult,
                        op1=OP.add,
                    )

                    # ---- batched stores (chunk-padded DRAM layout) ----
                    rp = row_base + b * K * 384
                    nc.scalar.dma_start(
                        out=boxes_d[rp : rp + K * 384, :].rearrange(
                            "(k p a) f -> p k a f", p=128, a=3
                        ),
                        in_=otile[:, :].rearrange("p (k a f) -> p k a f", a=3, f=6),
                    )
                    nc.scalar.dma_start(
                        out=valid_d[rp : rp + K * 384].rearrange(
                            "(k p a) -> p k a", p=128, a=3
                        ),
                        in_=vtile[:, :].rearrange("p (k a) -> p k a", a=3),
                    )
                row_base += BL * K * 384
    return nc


def run(output_13, output_26, output_52, anchors_13, anchors_26, anchors_52, thresh,
        **spmd_kwargs):
    """Builds + runs the 8-core kernel; returns ((boxes, valid), BassKernelResults)."""
    anchors = [np.asarray(anchors_13), np.asarray(anchors_26), np.asarray(anchors_52)]
    th = float(np.asarray(thresh))
    # sigmoid(v) > th  <=>  v > logit(th)
    logit_thresh = float(np.float32(math.log(th / (1.0 - th))))

    nc = bass.Bass()
    _trace_kernel(nc, logit_thresh)
    _legalize_waits(nc)

    consts = _build_consts(anchors)
    raw = {13: output_13, 26: output_26, 52: output_52}
    feats = {}
    for side, _, hw, k in SCALES:
        f = np.asarray(raw[side]).reshape(B, CH, hw)
        fp = np.zeros((B, CH, k * 128), dtype=np.float32)
        fp[:, :, :hw] = f
        feats[side] = fp
    in_maps = []
    for g in range(NCORES):
        m = {f"x{side}": np.ascontiguousarray(feats[side][g * BL : (g + 1) * BL])
             for side, _, _, _ in SCALES}
        m.update(consts)
        in_maps.append(m)

    br = run_bass_kernel_spmd(nc, in_maps, list(range(NCORES)), **spmd_kwargs)
    res = br.results

    # host gather: per-scale blocks, cores (= batch shards) in order,
    # stripping the per-(batch,scale) chunk padding
    boxes_parts, valid_parts = [], []
    off = 0
    for side, _, hw, k in SCALES:
        nv = hw * 3
        for g in range(NCORES):
            for b in range(BL):
                r0 = off + b * k * 384
                boxes_parts.append(res[g]["boxes"][r0 : r0 + nv])
                valid_parts.append(res[g]["valid"][r0 : r0 + nv])
        off += BL * k * 384
    boxes = np.concatenate(boxes_parts, axis=0)
    valid = np.concatenate(valid_parts, axis=0) != 0
    return (boxes, valid), br


def kernel(output_13, output_26, output_52, anchors_13, anchors_26, anchors_52, thresh):
    (boxes, valid), _ = run(
        output_13, output_26, output_52, anchors_13, anchors_26, anchors_52, thresh
    )
    return boxes, valid


if __name__ == "__main__":
    rng = np.random.default_rng(0)
    ins = {
        "output_13": rng.standard_normal((B, CH, 13, 13), dtype=np.float32),
        "output_26": rng.standard_normal((B, CH, 26, 26), dtype=np.float32),
        "output_52": rng.standard_normal((B, CH, 52, 52), dtype=np.float32),
        "anchors_13": np.array([[116.0, 90.0], [156.0, 198.0], [373.0, 326.0]], np.float32),
        "anchors_26": np.array([[30.0, 61.0], [62.0, 45.0], [59.0, 119.0]], np.float32),
        "anchors_52": np.array([[10.0, 13.0], [16.0, 30.0], [33.0, 23.0]], np.float32),
        "thresh": np.float32(0.6),
    }
    b, v = kernel(**ins)
    print(b.shape, v.shape, v.sum())


# revision 28
# speedup vs baseline: 1.0799x; 1.0538x over previous
"""YOLO-style detector decode kernel for Trainium2, 8-core data-parallel.

kernel(**inputs) takes the full unsharded inputs (as produced by
setup_inputs) and returns (boxes [N,6] f32, valid [N] bool), matching the
reference. Batch is sharded 4-per-core across 8 NeuronCores; each core
decodes its 4 batches for all three scales with no cross-core
communication; host re-concatenates per-scale blocks.

Per-core pipeline (per batch x scale group):
  - DMA in two channel granules [128, HW] + [127, HW] (padded to K*128
    spatial cols)
  - PE transposes each 128-spatial chunk into PSUM [128, 255]
    (channels on the free axis)
  - class max m via DVE reduce_max on the transposed tiles (batched over
    4-chunk PSUM tiles)
  - DVE scalar_tensor_tensor per (chunk, anchor): (v >= m) * 2^-c,
    accum-summed -> x whose float32 exponent encodes the first argmax
    index exactly (ties sum into lower-order bits, leading exponent stays
    at the smallest class index)
  - cls = -((bitcast_i32(x) >> 23) - 127)
  - box fields: batched ACT sigmoid/exp + batched DVE ops build
    [conf, x1, y1, x2, y2, cls]; objectness mask compares raw logits
    against logit(thresh) so the bool mask is bit-exact vs the reference
"""

import math
import sys

import numpy as np

for _p in ("/opt/trn_rl_repo",):
    if _p not in sys.path:
        sys.path.insert(0, _p)

import bass_rust
import concourse.bass as bass
import concourse.mybir as mybir
from concourse.bass_utils import run_bass_kernel_spmd
from concourse.tile import TileContext

F32 = mybir.dt.float32
BF16 = mybir.dt.bfloat16
I32 = mybir.dt.int32
U8 = mybir.dt.uint8
BIGSCALE = float(2.0 ** 120)
# ACT-side extraction route: net-negative on the cost model (accumulate
# still runs on DVE); keep the pure-DVE path
ACT_ROUTE = False
# chunks per PSUM tile (tile free size = CPT*256 f32); PSUM bufs chosen so
# CPT * bufs * 256 * 4B <= 16KB/partition
CPT = 2
PSUM_BUFS = 8

B = 32
NCORES = 8
BL = B // NCORES  # 4 batches per core
CH = 255
NCLS = 80
# (side, stride, HW, K chunks of 128 spatial)
SCALES = [(52, 8.0, 2704, 22), (26, 16.0, 676, 6), (13, 32.0, 169, 2)]
ROWS_PER_CORE = BL * sum(hw for _, _, hw, _ in SCALES) * 3  # 42588
# DRAM outputs are padded to whole 128-spatial chunks per (batch, scale)
ROWS_PAD = BL * sum(k * 384 for _, _, _, k in SCALES)  # 46080


def _build_consts(anchors_by_scale):
    """Host-side constant tensors shared by all cores."""
    consts = {}
    consts["ident"] = np.eye(128, dtype=np.float32)
    # 2^-c weights, replicated on all 128 partitions
    w = (2.0 ** -np.arange(NCLS, dtype=np.float64)).astype(np.float32)
    consts["wexact"] = np.broadcast_to(w, (128, NCLS)).copy()
    import ml_dtypes
    consts["wexb"] = consts["wexact"].astype(ml_dtypes.bfloat16)
    for side, t, hw, k in SCALES:
        g = np.zeros((128, k * 6), dtype=np.float32)
        awh = np.zeros((128, k * 6), dtype=np.float32)
        anc = anchors_by_scale[side]  # [3,2]
        for kk in range(k):
            for p in range(128):
                s = kk * 128 + p
                if s >= hw:
                    continue
                gx = float(s % side) * t
                gy = float(s // side) * t
                for a in range(3):
                    g[p, kk * 6 + a * 2 + 0] = gx
                    g[p, kk * 6 + a * 2 + 1] = gy
        for a in range(3):
            awh[:, a * 2 + 0 :: 6] = float(anc[a, 0]) * 0.5
            awh[:, a * 2 + 1 :: 6] = float(anc[a, 1]) * 0.5
        consts[f"grid{side}"] = g
        consts[f"awh{side}"] = awh
    return consts


def _legalize_waits(nc, maxw=1):
    """HW ISA instructions have a small number of sync-wait slots; Tile can
    emit more. Hoist excess waits onto same-engine NoOps placed immediately
    before the instruction (engine program order preserves semantics)."""
    n = 0
    for fn in nc.m.functions:
        for blk in fn.blocks:
            li = blk.instructions
            out = []
            for inst in li:
                maxw_i = maxw
                si = inst.sync_info
                waits = list(si.on_wait) if si is not None and si.on_wait else []
                if len(waits) > maxw_i:
                    hoisted, keep = waits[: len(waits) - maxw_i], waits[-maxw_i:]
                    for j in range(0, len(hoisted), maxw):
                        nop = bass_rust.InstNoOp(
                            name=f"waitnop-{n}", ins=[], outs=[]
                        )
                        n += 1
                        nop.engine = inst.engine
                        nop.sync_info = mybir.SyncInfo(
                            on_wait=hoisted[j : j + maxw], on_update=[]
                        )
                        out.append(nop)
                    inst.sync_info = mybir.SyncInfo(
                        on_wait=keep,
                        on_update=list(si.on_update) if si.on_update else [],
                    )
                out.append(inst)
            li[:] = out


def _trace_kernel(nc, logit_thresh):
    """Emit the per-core BIR program. All cores run the same program."""
    xs = {}
    for side, _, hw, k in SCALES:
        # inputs are host-padded to whole 128-spatial chunks
        xs[side] = nc.declare_dram_parameter(f"x{side}", [BL, CH, k * 128], F32, isOutput=False)
    ident_d = nc.declare_dram_parameter("ident", [128, 128], F32, isOutput=False)
    wexact_d = nc.declare_dram_parameter("wexact", [128, NCLS], F32, isOutput=False)
    wexb_d = nc.declare_dram_parameter("wexb", [128, NCLS], BF16, isOutput=False)
    grid_d, awh_d = {}, {}
    for side, _, hw, k in SCALES:
        grid_d[side] = nc.declare_dram_parameter(f"grid{side}", [128, k * 6], F32, isOutput=False)
        awh_d[side] = nc.declare_dram_parameter(f"awh{side}", [128, k * 6], F32, isOutput=False)
    boxes_d = nc.declare_dram_parameter("boxes", [ROWS_PAD, 6], F32, isOutput=True)
    valid_d = nc.declare_dram_parameter("valid", [ROWS_PAD], U8, isOutput=True)

    AX = mybir.AxisListType.X
    OP = mybir.AluOpType
    ACTF = mybir.ActivationFunctionType

    with TileContext(nc) as tc:
        with (
            tc.tile_pool(name="consts", bufs=1) as cpool,
            tc.tile_pool(name="gran", bufs=3) as gpool,
            tc.tile_pool(name="work", bufs=3) as wpool,
            tc.tile_pool(name="scr", bufs=4) as spool,
            tc.tile_pool(name="psum", bufs=PSUM_BUFS, space="PSUM") as ppool,
        ):
            ident = cpool.tile([128, 128], F32, tag="ident")
            nc.sync.dma_start(out=ident[:, :], in_=ident_d[:, :])
            wexact = cpool.tile([128, NCLS], F32, tag="wex")
            nc.sync.dma_start(out=wexact[:, :], in_=wexact_d[:, :])
            wexb = cpool.tile([128, NCLS], BF16, tag="wexb")
            nc.sync.dma_start(out=wexb[:, :], in_=wexb_d[:, :])
            grids, awhs = {}, {}
            for side, _, hw, k in SCALES:
                grids[side] = cpool.tile(
                    [128, k * 6], F32, tag=f"grid{side}", name=f"grid{side}"
                )
                nc.sync.dma_start(out=grids[side][:, :], in_=grid_d[side][:, :])
                awhs[side] = cpool.tile(
                    [128, k * 6], F32, tag=f"awh{side}", name=f"awh{side}"
                )
                nc.sync.dma_start(out=awhs[side][:, :], in_=awh_d[side][:, :])

            row_base = 0  # running row offset into the per-core output
            for side, t, hw, K in SCALES:
                KP = K * 128
                for b in range(BL):
                    # --- channel-major granules (input pre-padded on host) ---
                    g0 = gpool.tile([128, KP], F32, tag=f"g0_{side}")
                    g1 = gpool.tile([127, KP], F32, tag=f"g1_{side}")
                    nc.sync.dma_start(out=g0[:, :], in_=xs[side][b, 0:128, :])
                    nc.sync.dma_start(out=g1[:, :], in_=xs[side][b, 128:255, :])

                    sxyc = wpool.tile([128, K * 9], F32, tag=f"sxyc{side}")
                    ewh = wpool.tile([128, K * 6], F32, tag=f"ewh{side}")
                    cxy = wpool.tile([128, K * 6], F32, tag=f"cxy{side}")
                    mmax = wpool.tile([128, K * 3], F32, tag=f"m{side}")
                    negm2 = wpool.tile([128, K * 3], F32, tag=f"nm{side}")
                    xacc = wpool.tile([128, K * 3], F32, tag=f"x{side}")
                    clsi = wpool.tile([128, K * 3], I32, tag=f"ci{side}")
                    otile = wpool.tile([128, K * 18], F32, tag=f"o{side}")
                    vtile = wpool.tile([128, K * 3], U8, tag=f"v{side}")

                    nblk = (K + CPT - 1) // CPT
                    for bk in range(nblk):
                        k0 = bk * CPT
                        nb = min(CPT, K - k0)
                        tp = ppool.tile([128, CPT * 256], F32, tag="tp")
                        for kk in range(k0, k0 + nb):
                            cb = (kk - k0) * 256
                            s0 = kk * 128
                            sl = slice(s0, s0 + 128)
                            nc.tensor.transpose(
                                tp[:, cb : cb + 128], g0[:, sl], ident[0:128, 0:128]
                            )
                            nc.tensor.transpose(
                                tp[:, cb + 128 : cb + 255], g1[:, sl], ident[0:127, 0:127]
                            )
                        tca = tp[:, :].rearrange("p (c x) -> p c x", c=CPT)
                        t4 = tca[:, 0:nb, 0:255].rearrange(
                            "p c (a f) -> p c a f", a=3
                        )  # [128,nb,3,85]
                        tbox = t4[:, :, :, 0:5]
                        tcls = t4[:, :, :, 5:85]
                        m3 = mmax[:, :].rearrange("p (k a) -> p k a", a=3)
                        nc.vector.reduce_max(
                            out=m3[:, k0 : k0 + nb, :], in_=tcls, axis=AX
                        )
                        if ACT_ROUTE and side == 52:
                            # bias for the ACT-side extraction: -m * 2^120
                            nc.vector.tensor_scalar(
                                out=negm2[:, k0 * 3 : (k0 + nb) * 3],
                                in0=mmax[:, k0 * 3 : (k0 + nb) * 3],
                                scalar1=-BIGSCALE,
                                scalar2=None,
                                op0=OP.mult,
                            )
                        # objectness from raw logits, batched over the block
                        nc.vector.tensor_scalar(
                            out=vtile[:, :].rearrange("p (k a) -> p k a", a=3)[
                                :, k0 : k0 + nb, :
                            ],
                            in0=tbox[:, :, :, 0:1],
                            scalar1=logit_thresh,
                            scalar2=None,
                            op0=OP.is_gt,
                        )
                        # sigmoid of (conf, tx, ty); exp of (tw, th)
                        nc.scalar.activation(
                            out=sxyc[:, :].rearrange("p (k a d) -> p k a d", a=3, d=3)[
                                :, k0 : k0 + nb, :, :
                            ],
                            in_=tbox[:, :, :, 0:3],
                            func=ACTF.Sigmoid,
                        )
                        nc.scalar.activation(
                            out=ewh[:, :].rearrange("p (k a d) -> p k a d", a=3, d=2)[
                                :, k0 : k0 + nb, :, :
                            ],
                            in_=tbox[:, :, :, 3:5],
                            func=ACTF.Exp,
                        )
                        # (v >= m) * 2^-c, accum -> exponent encodes argmax
                        for kk in range(k0, k0 + nb):
                            cb = (kk - k0) * 256
                            for a in range(3):
                                col = kk * 3 + a
                                if ACT_ROUTE and side == 52 and a >= 1:
                                    # exact ACT route: q = sigmoid((v-m)*2^120)
                                    # is exactly 0.5 at the max, 0 elsewhere;
                                    # 2q * 2^-c accumulated on GPSIMD
                                    q = spool.tile(
                                        [128, NCLS], BF16, tag="q", name="q"
                                    )
                                    nc.scalar.activation(
                                        out=q[:, :],
                                        in_=tp[:, cb + 85 * a + 5 : cb + 85 * a + 85],
                                        func=ACTF.Sigmoid,
                                        bias=negm2[:, col : col + 1],
                                        scale=BIGSCALE,
                                    )
                                    scr2 = spool.tile(
                                        [128, NCLS], BF16, tag="scr2", name="scr2"
                                    )
                                    nc.vector.scalar_tensor_tensor(
                                        out=scr2[:, :],
                                        in0=q[:, :],
                                        scalar=2.0,
                                        in1=wexb[:, :],
                                        op0=OP.mult,
                                        op1=OP.mult,
                                        accum_out=xacc[:, col : col + 1],
                                    )
                                else:
                                    scr = spool.tile([128, NCLS], F32, tag="scr")
                                    nc.vector.scalar_tensor_tensor(
                                        out=scr[:, :],
                                        in0=tp[:, cb + 85 * a + 5 : cb + 85 * a + 85],
                                        scalar=mmax[:, col : col + 1],
                                        in1=wexact[:, :],
                                        op0=OP.is_ge,
                                        op1=OP.mult,
                                        accum_out=xacc[:, col : col + 1],
                                    )

                    # ---- batched per-group ops ----
                    s4 = sxyc[:, :].rearrange("p (k a d) -> p k a d", a=3, d=3)
                    o4 = otile[:, :].rearrange("p (k a f) -> p k a f", a=3, f=6)
                    c4 = cxy[:, :].rearrange("p (k a d) -> p k a d", a=3, d=2)
                    e4 = ewh[:, :].rearrange("p (k a d) -> p k a d", a=3, d=2)
                    # conf -> output
                    nc.gpsimd.tensor_copy(o4[:, :, :, 0:1], s4[:, :, :, 0:1])
                    # half extents: exp * (anchor/2)
                    nc.vector.tensor_tensor(
                        out=ewh[:, :], in0=ewh[:, :], in1=awhs[side][:, :], op=OP.mult
                    )
                    # centers: sig*t + grid*t
                    nc.vector.scalar_tensor_tensor(
                        out=cxy[:, :],
                        in0=s4[:, :, :, 1:3],
                        scalar=float(t),
                        in1=grids[side][:, :].rearrange(
                            "p (k a d) -> p k a d", a=3, d=2
                        ),
                        op0=OP.mult,
                        op1=OP.add,
                    )
                    nc.vector.tensor_tensor(
                        out=o4[:, :, :, 1:3], in0=c4, in1=e4, op=OP.subtract
                    )
                    nc.vector.tensor_tensor(
                        out=o4[:, :, :, 3:5], in0=c4, in1=e4, op=OP.add
                    )
                    # cls decode: exponent bits of xacc
                    nc.vector.tensor_scalar(
                        out=clsi[:, :],
                        in0=xacc[:, :].bitcast(I32),
                        scalar1=23,
                        scalar2=None,
                        op0=OP.logical_shift_right,
                    )
                    # cls = 127 - biased_exponent
                    nc.vector.tensor_scalar(
                        out=o4[:, :, :, 5:6],
                        in0=clsi[:, :].rearrange("p (k a) -> p k a", a=3).unsqueeze(3),
                        scalar1=-1,
                        scalar2=127,
                        op0=OP.mult,
                        op1=OP.add,
                    )

                    # ---- batched stores (chunk-padded DRAM layout) ----
                    rp = row_base + b * K * 384
                    nc.scalar.dma_start(
                        out=boxes_d[rp : rp + K * 384, :].rearrange(
                            "(k p a) f -> p k a f", p=128, a=3
                        ),
                        in_=otile[:, :].rearrange("p (k a f) -> p k a f", a=3, f=6),
                    )
                    nc.scalar.dma_start(
                        out=valid_d[rp : rp + K * 384].rearrange(
                            "(k p a) -> p k a", p=128, a=3
                        ),
                        in_=vtile[:, :].rearrange("p (k a) -> p k a", a=3),
                    )
                row_base += BL * K * 384
    return nc


def run(output_13, output_26, output_52, anchors_13, anchors_26, anchors_52, thresh,
        **spmd_kwargs):
    """Builds + runs the 8-core kernel; returns ((boxes, valid), BassKernelResults)."""
    anchors = {13: np.asarray(anchors_13), 26: np.asarray(anchors_26), 52: np.asarray(anchors_52)}
    th = float(np.asarray(thresh))
    # sigmoid(v) > th  <=>  v > logit(th)
    logit_thresh = float(np.float32(math.log(th / (1.0 - th))))

    nc = bass.Bass()
    _trace_kernel(nc, logit_thresh)
    _legalize_waits(nc)

    consts = _build_consts(anchors)
    raw = {13: output_13, 26: output_26, 52: output_52}
    feats = {}
    for side, _, hw, k in SCALES:
        f = np.asarray(raw[side]).reshape(B, CH, hw)
        fp = np.zeros((B, CH, k * 128), dtype=np.float32)
        fp[:, :, :hw] = f
        feats[side] = fp
    in_maps = []
    for g in range(NCORES):
        m = {f"x{side}": np.ascontiguousarray(feats[side][g * BL : (g + 1) * BL])
             for side, _, _, _ in SCALES}
        m.update(consts)
        in_maps.append(m)

    br = run_bass_kernel_spmd(nc, in_maps, list(range(NCORES)), **spmd_kwargs)
    res = br.results

    # host gather: reference output order is scale 13, 26, 52; per-core
    # blocks live at emission-order offsets. Strip the chunk padding.
    offs, off = {}, 0
    for side, _, hw, k in SCALES:
        offs[side] = off
        off += BL * k * 384
    byside = {side: (hw, k) for side, _, hw, k in SCALES}
    boxes_parts, valid_parts = [], []
    for side in (13, 26, 52):
        hw, k = byside[side]
        nv = hw * 3
        for g in range(NCORES):
            for b in range(BL):
                r0 = offs[side] + b * k * 384
                boxes_parts.append(res[g]["boxes"][r0 : r0 + nv])
                valid_parts.append(res[g]["valid"][r0 : r0 + nv])
    boxes = np.concatenate(boxes_parts, axis=0)
    valid = np.concatenate(valid_parts, axis=0) != 0
    return (boxes, valid), br


def kernel(output_13, output_26, output_52, anchors_13, anchors_26, anchors_52, thresh):
    (boxes, valid), _ = run(
        output_13, output_26, output_52, anchors_13, anchors_26, anchors_52, thresh
    )
    return boxes, valid


if __name__ == "__main__":
    rng = np.random.default_rng(0)
    ins = {
        "output_13": rng.standard_normal((B, CH, 13, 13), dtype=np.float32),
        "output_26": rng.standard_normal((B, CH, 26, 26), dtype=np.float32),
        "output_52": rng.standard_normal((B, CH, 52, 52), dtype=np.float32),
        "anchors_13": np.array([[116.0, 90.0], [156.0, 198.0], [373.0, 326.0]], np.float32),
        "anchors_26": np.array([[30.0, 61.0], [62.0, 45.0], [59.0, 119.0]], np.float32),
        "anchors_52": np.array([[10.0, 13.0], [16.0, 30.0], [33.0, 23.0]], np.float32),
        "thresh": np.float32(0.6),
    }
    b, v = kernel(**ins)
    print(b.shape, v.shape, v.sum())
